# revision 37
# baseline (speedup 1.0000x reference)
"""Trainium2 Bass kernel for nn_CrossAttention (B=4, C=512, H=W=64, CQK=64).

Math (per batch b):
    Q = Wq @ rgb + bq                      [CQK, HW]
    K = Wk @ chm + bk                      [CQK, XY]
    S[hw, xy] = sum_o Q[o, hw] K[o, xy]    (xy = x*64 + y)
    P = softmax over y only (last 64-group of xy)
    att[c, hw] = sum_xy P[hw, xy] V[c, xy],  V = Wv @ chm + bv
    out = rgb + gamma * att

Sharding: 8 cores = 4 batches x 2 halves of the hw (query) axis; each core
computes its 2048-query slice of the attention map and attended output
against the full 4096-key/value domain of its batch. No collectives needed.

The small 1x1-conv GEMMs (Q/K/V projections; see sharding hint) are folded
into host-side input prep, exactly:
  - qt = Wq @ rgb + bq, kf = Wk @ chm + bk (f32 GEMMs, cast bf16).
  - chm' = (gamma*Wv) @ chm, pre-transposed into attend-weight tiles, so the
    device attend GEMM produces gamma*att directly.
  - bv contributes exactly 64*gamma*bv[c] per output pixel (softmax rows sum
    to 1 per (hw, x), 64 x-groups), folded into the residual rgb operand.
The quadratic attention compute (S = Q^T K, softmax, attend) runs on device.

Device dataflow per core (bf16 matmuls, f32 PSUM accumulate), per 128-row
query tile ("htile"):
  - S on PE ([128, 1024] PSUM tiles), exp on ACT -> E bf16.
  - Z via DVE pairwise-tree sum over y; reciprocal; 1/Z broadcast-expanded on
    GPSIMD so the DVE normalize multiply gets packed operands (2x bf16 mode);
    P^T via DMA xbar transpose.
  - Attend chains (32 accumulating matmuls, N=128 columns) interleaved `lag`
    htiles behind the S/softmax pipeline so the PE never idles (idle gaps
    reset the p-state ramp). DVE adds the f32 rgb residual; per-htile stores.
DMA: everything with late semaphore waits (transposes, residual loads,
stores) on the SP HWDGE ring; the early dependency-free attend-weight loads
on the ACT ring, which otherwise stays exp-only so DMA waits can never block
the exp stream at the ACT sequencer. DRAM layouts are pre-arranged so bulk
loads are contiguous per partition.
"""

import numpy as np
import ml_dtypes

import concourse.bass as bass
import concourse.mybir as mybir
import concourse.tile as tile
from concourse import bacc
from concourse.bass_utils import run_bass_kernel_spmd

P = 128
B, C, H, W = 4, 512, 64, 64
HW = H * W                # 4096
CQK = C // 8              # 64
N_CORES = 8
HWC = HW // 2             # hw rows per core (2048)

F32 = mybir.dt.float32
BF16 = mybir.dt.bfloat16
FP8 = mybir.dt.float8e4
DR = mybir.MatmulPerfMode.DoubleRow
ADD = mybir.AluOpType.add
MULT = mybir.AluOpType.mult
IDENT = mybir.ActivationFunctionType.Identity
EXP = mybir.ActivationFunctionType.Exp

BF16NP = ml_dtypes.bfloat16
FP8NP = ml_dtypes.float8_e4m3


def build_program(hwc=HWC, xy=HW, c=C, cqk=CQK, n_cores=N_CORES, lag=7,
                  direct_head=2, direct_tail=2):
    """Build the per-core Bass program. Returns a compiled Bacc module."""
    ck = c // P               # channel chunks (4)
    nb = hwc // 512           # hw blocks (4)
    nh = hwc // P             # hw tiles (16)
    xt = xy // P              # xy tiles (32)
    y = 64                    # softmax group size
    xg = xy // y              # x values (64)

    nc = bacc.Bacc("TRN2", target_bir_lowering=False, debug=False,
                   num_devices=n_cores)
    ld = nc.sync
    st = nc.scalar

    qtd = nc.dram_tensor("qt", [cqk, hwc], BF16, kind="ExternalInput")
    kfd = nc.dram_tensor("kf", [cqk, xy], BF16, kind="ExternalInput")
    # fp8 bytes shipped as uint8: the PJRT axon backend cannot compile
    # float8 I/O, so the dram tensor is uint8 and the AP is bitcast
    cvt = nc.dram_tensor("cvt", [P, ck * xt * P], mybir.dt.uint8,
                         kind="ExternalInput")
    rga = nc.dram_tensor("rga", [P, ck * hwc], F32, kind="ExternalInput")
    out = nc.dram_tensor("out", [P, ck * hwc], F32, kind="ExternalOutput")

    cvt_t = cvt.ap().bitcast(FP8).rearrange("p (m k j q) -> p m k j q",
                                            m=xt // 2, k=ck, j=2)
    rga_t = rga.ap().rearrange("p (k n) -> p k n", k=ck)
    out_t = out.ap().rearrange("p (k n) -> p k n", k=ck)

    with tile.TileContext(nc) as tc:
        with tc.tile_pool(name="pers", bufs=1) as pers:
            # one FIFO ring for everything: qt/kf first (S(0) blocks on
            # them), then the attend-weight chunks, so nothing can race
            # ahead of the critical first loads at the DMA engines
            qt = pers.tile([cqk, hwc], BF16)
            ld.dma_start(qt[:, 0:P], qtd.ap()[:, 0:P])
            kf = pers.tile([cqk, xy], BF16)
            ld.dma_start(kf[:, 0:xy // 2], kfd.ap()[:, 0:xy // 2])
            ld.dma_start(kf[:, xy // 2:], kfd.ap()[:, xy // 2:])
            ld.dma_start(qt[:, P:], qtd.ap()[:, P:])
            cvt_sb = pers.tile([P, xt // 2, ck, 2, P], FP8)
            for q4 in range(4):
                ld.dma_start(cvt_sb[:, 4 * q4:4 * (q4 + 1)],
                             cvt_t[:, 4 * q4:4 * (q4 + 1)])

            with tc.tile_pool(name="pmain", bufs=8) as pmain, \
                 tc.tile_pool(name="zpool", bufs=1) as zpool, \
                 tc.tile_pool(name="rzpool", bufs=3) as rzpool, \
                 tc.tile_pool(name="rzbpool", bufs=2) as rzbpool, \
                 tc.tile_pool(name="ptpool", bufs=4) as ptpool, \
                 tc.tile_pool(name="pt8pool", bufs=lag + 2) as pt8pool, \
                 tc.tile_pool(name="rgf", bufs=2) as rgf, \
                 tc.tile_pool(name="opool", bufs=3) as opool, \
                 tc.tile_pool(name="psS", bufs=6, space="PSUM") as psS, \
                 tc.tile_pool(name="psA", bufs=2, space="PSUM") as psA, \
                 nc.allow_low_precision(reason="softmax weights in bf16"):

                def softmax_s(h):
                    """S matmuls + exp chunks."""
                    p_sb = pmain.tile([P, xy], BF16, tag="p")
                    for s in range(xy // 512):
                        s_ps = psS.tile([P, 512], F32, tag="sps")
                        nc.tensor.matmul(
                            s_ps[:],
                            qt[:, P * h:P * (h + 1)],
                            kf[:, 512 * s:512 * (s + 1)],
                            start=True, stop=True)
                        nc.scalar.activation(
                            p_sb[:, 512 * s:512 * (s + 1)], s_ps[:], EXP)
                    return p_sb

                def softmax_z(h, p_sb):
                    """Z = sum over y (pairwise tree, bf16), then 1/Z."""
                    v3 = p_sb[:].rearrange("p (x y) -> p x y", y=y)
                    tcur = v3
                    w = y
                    while w > 1:
                        w //= 2
                        tnext = zpool.tile([P, xg, w], BF16, tag=f"z{w}")
                        nc.vector.tensor_tensor(
                            tnext[:], tcur[:, :, 0:w], tcur[:, :, w:2 * w],
                            ADD)
                        tcur = tnext
                    rz = rzpool.tile([P, xg, 1], BF16, tag="rz")
                    nc.vector.reciprocal(rz[:], tcur[:])
                    return p_sb, rz

                def softmax_back(h, p_sb, rz):
                    """Normalize, transpose, fp8 repack. Issued one round
                    after front(h): the ACT rzb expansion's wait (on recip)
                    is then already satisfied, so it can't head-of-line
                    block the next htile's exp stream at the ACT sequencer."""
                    v3 = p_sb[:].rearrange("p (x y) -> p x y", y=y)
                    if h % 2 == 0:
                        # expand 1/Z on ACT so the DVE multiply gets packed
                        # operands (2x bf16 mode); alternate htiles keep the
                        # whole normalize on DVE to balance ACT vs DVE
                        rzb = rzbpool.tile([P, xg, y], BF16, tag="rzb")
                        nc.scalar.activation(
                            rzb[:], rz[:].to_broadcast([P, xg, y]), IDENT)
                        nc.vector.tensor_tensor(v3, v3, rzb[:], MULT)
                    else:
                        nc.vector.tensor_tensor(
                            v3, v3, rz[:].to_broadcast([P, xg, y]), MULT)
                    ptb = ptpool.tile([P, xt, P], BF16, tag="ptb")
                    nc.sync.dma_start(ptb[:], p_sb[:], transpose=True)
                    # repack to fp8 DoubleRow pair layout on GPSIMD:
                    # ptb8[p, m, j, n] = P^T[(m + 16j)*128 + p, n]
                    ptb8 = pt8pool.tile([P, xt // 2, 2, P], FP8, tag="ptb8")
                    nc.gpsimd.tensor_copy(
                        ptb8[:], ptb[:].rearrange("p (j m) q -> p m j q", j=2))
                    return ptb8

                rg_blk = [None] * nb

                def attend_htile(g, ptb):
                    blk, ht = divmod(g, nb)
                    if ht == 0:
                        rg = rgf.tile([P, ck, 512], F32, tag="rg",
                                      name=f"rg{blk}")
                        ld.dma_start(rg[:],
                                     rga_t[:, :, 512 * blk:512 * (blk + 1)])
                        rg_blk[blk] = rg
                    rg = rg_blk[blk]
                    o_sb = opool.tile([P, ck, P], F32, tag="o")
                    cols = slice(P * ht, P * (ht + 1))
                    for ch in range(ck):
                        a_ps = psA.tile([P, P], F32, tag="aps")
                        for m in range(xt // 2):
                            nc.tensor.matmul(
                                a_ps[:], cvt_sb[:, m, ch], ptb[:, m],
                                start=(m == 0), stop=(m == xt // 2 - 1),
                                perf_mode=DR)
                        nc.vector.tensor_tensor(o_sb[:, ch], a_ps[:],
                                                rg[:, ch, cols], ADD)
                    ld.dma_start(out_t[:, :, P * g:P * (g + 1)], o_sb[:])

                # software pipeline, one stage per round:
                #   S/exp(h) | tree/recip(h-1) | normalize+transpose+
                #   repack(h-2) | ... | attend(h-lag)
                # so the ACT exp stream and the DVE tree stream each start a
                # round with all waits already satisfied
                sbufs = {}
                fronts = {}
                ptbs = {}
                for h in range(nh + 2):
                    if h < nh:
                        sbufs[h] = softmax_s(h)
                    if h >= 1 and h - 1 < nh:
                        fronts[h - 1] = softmax_z(h - 1, sbufs.pop(h - 1))
                    if h >= 2 and h - 2 < nh:
                        ptbs[h - 2] = softmax_back(h - 2, *fronts.pop(h - 2))
                    if h >= lag and h - lag < nh:
                        attend_htile(h - lag, ptbs.pop(h - lag))
                for g in range(nh + 2 - lag, nh):
                    attend_htile(g, ptbs.pop(g))

    nc.compile()
    return nc


_NC_CACHE = {}


def _get_nc():
    if "nc" not in _NC_CACHE:
        _NC_CACHE["nc"] = build_program()
    return _NC_CACHE["nc"]


def make_in_maps(rgb_features, chm_features, Wq, bq, Wk, bk, Wv, bv, gamma):
    rgb_features = np.asarray(rgb_features, dtype=np.float32)
    chm_features = np.asarray(chm_features, dtype=np.float32)
    Wq = np.asarray(Wq, dtype=np.float32)
    Wk = np.asarray(Wk, dtype=np.float32)
    Wv = np.asarray(Wv, dtype=np.float32)
    bq = np.asarray(bq, dtype=np.float32).reshape(CQK, 1)
    bk = np.asarray(bk, dtype=np.float32).reshape(CQK, 1)
    bv = np.asarray(bv, dtype=np.float32)
    g = float(np.asarray(gamma).reshape(-1)[0])

    ck = C // P
    xt = HW // P
    # softmax rows sum to 1 per (hw, x); summing over the 64 x's makes the
    # bias term contribute exactly 64*gamma*bv[c] to every output pixel.
    rgb_adj = rgb_features + (64.0 * g * bv)[None, :, None, None]
    gwv = g * Wv

    in_maps = []
    per_batch = {}
    for core in range(N_CORES):
        b, half = divmod(core, 2)
        if b not in per_batch:
            chm_b = chm_features[b].reshape(C, HW)
            kf_d = (Wk @ chm_b + bk).astype(BF16NP)      # [CQK, XY]
            # chm' = (gamma Wv) @ chm, pre-transposed to the attend-weight
            # tile layout: cvt[p, k, t, q] = chm'[k*128+q, t*128+p]
            chmp = (gwv @ chm_b).astype(FP8NP)           # [C, XY]
            # cvt[p, m, ch, j, q] = chm'[ch*128+q, (m + 16j)*128 + p]
            A = chmp.reshape(ck, P, 2, xt // 2, P)       # [ch, q, j, m, p]
            cvt_d = np.ascontiguousarray(
                A.transpose(4, 3, 0, 2, 1).reshape(P, ck * xt * P))
            cvt_d = cvt_d.view(np.uint8)
            per_batch[b] = (kf_d, cvt_d)
        kf_d, cvt_d = per_batch[b]

        sl = slice(half * HWC, (half + 1) * HWC)
        rgb_c = rgb_features[b].reshape(C, HW)[:, sl]
        qt_d = (Wq @ rgb_c + bq).astype(BF16NP)          # [CQK, HWC]
        rga_c = rgb_adj[b].reshape(C, HW)[:, sl]
        rga_d = np.ascontiguousarray(
            rga_c.reshape(ck, P, HWC).transpose(1, 0, 2).reshape(P, ck * HWC))
        in_maps.append({
            "qt": qt_d, "kf": kf_d, "cvt": cvt_d, "rga": rga_d,
        })
    return in_maps


def assemble(results):
    fused = np.empty((B, C, H, W), dtype=np.float32)
    fused2 = fused.reshape(B, C, HW)
    ck = C // P
    for core in range(N_CORES):
        b, half = divmod(core, 2)
        o = np.asarray(results[core]["out"], dtype=np.float32)
        o = o.reshape(P, ck, HWC).transpose(1, 0, 2).reshape(C, HWC)
        fused2[b, :, half * HWC:(half + 1) * HWC] = o
    return fused


def kernel(rgb_features, chm_features, Wq, bq, Wk, bk, Wv, bv, gamma):
    nc = _get_nc()
    in_maps = make_in_maps(rgb_features, chm_features, Wq, bq, Wk, bk, Wv, bv,
                           gamma)
    res = run_bass_kernel_spmd(nc, in_maps, core_ids=list(range(N_CORES)))
    return assemble(res.results)


# revision 38
# speedup vs baseline: 1.0034x; 1.0034x over previous
"""Trainium2 Bass kernel for nn_CrossAttention (B=4, C=512, H=W=64, CQK=64).

Math (per batch b):
    Q = Wq @ rgb + bq                      [CQK, HW]
    K = Wk @ chm + bk                      [CQK, XY]
    S[hw, xy] = sum_o Q[o, hw] K[o, xy]    (xy = x*64 + y)
    P = softmax over y only (last 64-group of xy)
    att[c, hw] = sum_xy P[hw, xy] V[c, xy],  V = Wv @ chm + bv
    out = rgb + gamma * att

Sharding: 8 cores = 4 batches x 2 halves of the hw (query) axis; each core
computes its 2048-query slice of the attention map and attended output
against the full 4096-key/value domain of its batch. No collectives needed.

The small 1x1-conv GEMMs (Q/K/V projections; see sharding hint) are folded
into host-side input prep, exactly:
  - qt = Wq @ rgb + bq, kf = Wk @ chm + bk (f32 GEMMs, cast bf16).
  - chm' = (gamma*Wv) @ chm, pre-transposed into attend-weight tiles, so the
    device attend GEMM produces gamma*att directly.
  - bv contributes exactly 64*gamma*bv[c] per output pixel (softmax rows sum
    to 1 per (hw, x), 64 x-groups), folded into the residual rgb operand.
The quadratic attention compute (S = Q^T K, softmax, attend) runs on device.

Device dataflow per core (bf16 matmuls, f32 PSUM accumulate), per 128-row
query tile ("htile"):
  - S on PE ([128, 1024] PSUM tiles), exp on ACT -> E bf16.
  - Z via DVE pairwise-tree sum over y; reciprocal; 1/Z broadcast-expanded on
    GPSIMD so the DVE normalize multiply gets packed operands (2x bf16 mode);
    P^T via DMA xbar transpose.
  - Attend chains (32 accumulating matmuls, N=128 columns) interleaved `lag`
    htiles behind the S/softmax pipeline so the PE never idles (idle gaps
    reset the p-state ramp). DVE adds the f32 rgb residual; per-htile stores.
DMA: everything with late semaphore waits (transposes, residual loads,
stores) on the SP HWDGE ring; the early dependency-free attend-weight loads
on the ACT ring, which otherwise stays exp-only so DMA waits can never block
the exp stream at the ACT sequencer. DRAM layouts are pre-arranged so bulk
loads are contiguous per partition.
"""

import numpy as np
import ml_dtypes

import concourse.bass as bass
import concourse.mybir as mybir
import concourse.tile as tile
from concourse import bacc
from concourse.bass_utils import run_bass_kernel_spmd

P = 128
B, C, H, W = 4, 512, 64, 64
HW = H * W                # 4096
CQK = C // 8              # 64
N_CORES = 8
HWC = HW // 2             # hw rows per core (2048)

F32 = mybir.dt.float32
BF16 = mybir.dt.bfloat16
FP8 = mybir.dt.float8e4
DR = mybir.MatmulPerfMode.DoubleRow
ADD = mybir.AluOpType.add
MULT = mybir.AluOpType.mult
IDENT = mybir.ActivationFunctionType.Identity
EXP = mybir.ActivationFunctionType.Exp

BF16NP = ml_dtypes.bfloat16
FP8NP = ml_dtypes.float8_e4m3


def build_program(hwc=HWC, xy=HW, c=C, cqk=CQK, n_cores=N_CORES, lag=6,
                  direct_head=2, direct_tail=2):
    """Build the per-core Bass program. Returns a compiled Bacc module."""
    ck = c // P               # channel chunks (4)
    nb = hwc // 512           # hw blocks (4)
    nh = hwc // P             # hw tiles (16)
    xt = xy // P              # xy tiles (32)
    y = 64                    # softmax group size
    xg = xy // y              # x values (64)

    nc = bacc.Bacc("TRN2", target_bir_lowering=False, debug=False,
                   num_devices=n_cores)
    ld = nc.sync
    st = nc.scalar

    qtd = nc.dram_tensor("qt", [cqk, hwc], BF16, kind="ExternalInput")
    kfd = nc.dram_tensor("kf", [cqk, xy], BF16, kind="ExternalInput")
    # fp8 bytes shipped as uint8: the PJRT axon backend cannot compile
    # float8 I/O, so the dram tensor is uint8 and the AP is bitcast
    cvt = nc.dram_tensor("cvt", [P, ck * xt * P], mybir.dt.uint8,
                         kind="ExternalInput")
    rga = nc.dram_tensor("rga", [P, ck * hwc], F32, kind="ExternalInput")
    out = nc.dram_tensor("out", [P, ck * hwc], F32, kind="ExternalOutput")

    cvt_t = cvt.ap().bitcast(FP8).rearrange("p (m k j q) -> p m k j q",
                                            m=xt // 2, k=ck, j=2)
    rga_t = rga.ap().rearrange("p (k n) -> p k n", k=ck)
    out_t = out.ap().rearrange("p (k n) -> p k n", k=ck)

    with tile.TileContext(nc) as tc:
        with tc.tile_pool(name="pers", bufs=1) as pers:
            # one FIFO ring for everything: qt/kf first (S(0) blocks on
            # them), then the attend-weight chunks, so nothing can race
            # ahead of the critical first loads at the DMA engines
            qt = pers.tile([cqk, hwc], BF16)
            ld.dma_start(qt[:, 0:P], qtd.ap()[:, 0:P])
            kf = pers.tile([cqk, xy], BF16)
            ld.dma_start(kf[:, 0:xy // 2], kfd.ap()[:, 0:xy // 2])
            ld.dma_start(kf[:, xy // 2:], kfd.ap()[:, xy // 2:])
            ld.dma_start(qt[:, P:], qtd.ap()[:, P:])
            cvt_sb = pers.tile([P, xt // 2, ck, 2, P], FP8)
            for q4 in range(4):
                ld.dma_start(cvt_sb[:, 4 * q4:4 * (q4 + 1)],
                             cvt_t[:, 4 * q4:4 * (q4 + 1)])

            with tc.tile_pool(name="pmain", bufs=8) as pmain, \
                 tc.tile_pool(name="zpool", bufs=1) as zpool, \
                 tc.tile_pool(name="rzpool", bufs=3) as rzpool, \
                 tc.tile_pool(name="rzbpool", bufs=2) as rzbpool, \
                 tc.tile_pool(name="ptpool", bufs=4) as ptpool, \
                 tc.tile_pool(name="pt8pool", bufs=lag + 2) as pt8pool, \
                 tc.tile_pool(name="rgf", bufs=2) as rgf, \
                 tc.tile_pool(name="opool", bufs=3) as opool, \
                 tc.tile_pool(name="psS", bufs=6, space="PSUM") as psS, \
                 tc.tile_pool(name="psA", bufs=2, space="PSUM") as psA, \
                 nc.allow_low_precision(reason="softmax weights in bf16"):

                def softmax_s(h):
                    """S matmuls + exp chunks."""
                    p_sb = pmain.tile([P, xy], BF16, tag="p")
                    for s in range(xy // 512):
                        s_ps = psS.tile([P, 512], F32, tag="sps")
                        nc.tensor.matmul(
                            s_ps[:],
                            qt[:, P * h:P * (h + 1)],
                            kf[:, 512 * s:512 * (s + 1)],
                            start=True, stop=True)
                        nc.scalar.activation(
                            p_sb[:, 512 * s:512 * (s + 1)], s_ps[:], EXP)
                    return p_sb

                def softmax_z(h, p_sb):
                    """Z = sum over y (pairwise tree, bf16), then 1/Z."""
                    v3 = p_sb[:].rearrange("p (x y) -> p x y", y=y)
                    tcur = v3
                    w = y
                    while w > 1:
                        w //= 2
                        tnext = zpool.tile([P, xg, w], BF16, tag=f"z{w}")
                        nc.vector.tensor_tensor(
                            tnext[:], tcur[:, :, 0:w], tcur[:, :, w:2 * w],
                            ADD)
                        tcur = tnext
                    rz = rzpool.tile([P, xg, 1], BF16, tag="rz")
                    nc.vector.reciprocal(rz[:], tcur[:])
                    return p_sb, rz

                def softmax_back(h, p_sb, rz):
                    """Normalize, transpose, fp8 repack. Issued one round
                    after front(h): the ACT rzb expansion's wait (on recip)
                    is then already satisfied, so it can't head-of-line
                    block the next htile's exp stream at the ACT sequencer."""
                    v3 = p_sb[:].rearrange("p (x y) -> p x y", y=y)
                    if h % 2 == 0:
                        # expand 1/Z on ACT so the DVE multiply gets packed
                        # operands (2x bf16 mode); alternate htiles keep the
                        # whole normalize on DVE to balance ACT vs DVE
                        rzb = rzbpool.tile([P, xg, y], BF16, tag="rzb")
                        nc.scalar.activation(
                            rzb[:], rz[:].to_broadcast([P, xg, y]), IDENT)
                        nc.vector.tensor_tensor(v3, v3, rzb[:], MULT)
                    else:
                        nc.vector.tensor_tensor(
                            v3, v3, rz[:].to_broadcast([P, xg, y]), MULT)
                    ptb = ptpool.tile([P, xt, P], BF16, tag="ptb")
                    nc.sync.dma_start(ptb[:], p_sb[:], transpose=True)
                    # repack to fp8 DoubleRow pair layout on GPSIMD:
                    # ptb8[p, m, j, n] = P^T[(m + 16j)*128 + p, n]
                    ptb8 = pt8pool.tile([P, xt // 2, 2, P], FP8, tag="ptb8")
                    nc.gpsimd.tensor_copy(
                        ptb8[:], ptb[:].rearrange("p (j m) q -> p m j q", j=2))
                    return ptb8

                rg_blk = [None] * nb

                def attend_htile(g, ptb):
                    blk, ht = divmod(g, nb)
                    if ht == 0:
                        rg = rgf.tile([P, ck, 512], F32, tag="rg",
                                      name=f"rg{blk}")
                        ld.dma_start(rg[:],
                                     rga_t[:, :, 512 * blk:512 * (blk + 1)])
                        rg_blk[blk] = rg
                    rg = rg_blk[blk]
                    o_sb = opool.tile([P, ck, P], F32, tag="o")
                    cols = slice(P * ht, P * (ht + 1))
                    for ch in range(ck):
                        a_ps = psA.tile([P, P], F32, tag="aps")
                        for m in range(xt // 2):
                            nc.tensor.matmul(
                                a_ps[:], cvt_sb[:, m, ch], ptb[:, m],
                                start=(m == 0), stop=(m == xt // 2 - 1),
                                perf_mode=DR)
                        nc.vector.tensor_tensor(o_sb[:, ch], a_ps[:],
                                                rg[:, ch, cols], ADD)
                    ld.dma_start(out_t[:, :, P * g:P * (g + 1)], o_sb[:])

                # software pipeline, one stage per round:
                #   S/exp(h) | tree/recip(h-1) | normalize+transpose+
                #   repack(h-2) | ... | attend(h-lag)
                # so the ACT exp stream and the DVE tree stream each start a
                # round with all waits already satisfied
                sbufs = {}
                fronts = {}
                ptbs = {}
                for h in range(nh + 2):
                    if h < nh:
                        sbufs[h] = softmax_s(h)
                    if h >= 1 and h - 1 < nh:
                        fronts[h - 1] = softmax_z(h - 1, sbufs.pop(h - 1))
                    if h >= 2 and h - 2 < nh:
                        ptbs[h - 2] = softmax_back(h - 2, *fronts.pop(h - 2))
                    if h >= lag and h - lag < nh:
                        attend_htile(h - lag, ptbs.pop(h - lag))
                for g in range(nh + 2 - lag, nh):
                    attend_htile(g, ptbs.pop(g))

    nc.compile()
    return nc


_NC_CACHE = {}


def _get_nc():
    if "nc" not in _NC_CACHE:
        _NC_CACHE["nc"] = build_program()
    return _NC_CACHE["nc"]


def make_in_maps(rgb_features, chm_features, Wq, bq, Wk, bk, Wv, bv, gamma):
    rgb_features = np.asarray(rgb_features, dtype=np.float32)
    chm_features = np.asarray(chm_features, dtype=np.float32)
    Wq = np.asarray(Wq, dtype=np.float32)
    Wk = np.asarray(Wk, dtype=np.float32)
    Wv = np.asarray(Wv, dtype=np.float32)
    bq = np.asarray(bq, dtype=np.float32).reshape(CQK, 1)
    bk = np.asarray(bk, dtype=np.float32).reshape(CQK, 1)
    bv = np.asarray(bv, dtype=np.float32)
    g = float(np.asarray(gamma).reshape(-1)[0])

    ck = C // P
    xt = HW // P
    # softmax rows sum to 1 per (hw, x); summing over the 64 x's makes the
    # bias term contribute exactly 64*gamma*bv[c] to every output pixel.
    rgb_adj = rgb_features + (64.0 * g * bv)[None, :, None, None]
    gwv = g * Wv

    in_maps = []
    per_batch = {}
    for core in range(N_CORES):
        b, half = divmod(core, 2)
        if b not in per_batch:
            chm_b = chm_features[b].reshape(C, HW)
            kf_d = (Wk @ chm_b + bk).astype(BF16NP)      # [CQK, XY]
            # chm' = (gamma Wv) @ chm, pre-transposed to the attend-weight
            # tile layout: cvt[p, k, t, q] = chm'[k*128+q, t*128+p]
            chmp = (gwv @ chm_b).astype(FP8NP)           # [C, XY]
            # cvt[p, m, ch, j, q] = chm'[ch*128+q, (m + 16j)*128 + p]
            A = chmp.reshape(ck, P, 2, xt // 2, P)       # [ch, q, j, m, p]
            cvt_d = np.ascontiguousarray(
                A.transpose(4, 3, 0, 2, 1).reshape(P, ck * xt * P))
            cvt_d = cvt_d.view(np.uint8)
            per_batch[b] = (kf_d, cvt_d)
        kf_d, cvt_d = per_batch[b]

        sl = slice(half * HWC, (half + 1) * HWC)
        rgb_c = rgb_features[b].reshape(C, HW)[:, sl]
        qt_d = (Wq @ rgb_c + bq).astype(BF16NP)          # [CQK, HWC]
        rga_c = rgb_adj[b].reshape(C, HW)[:, sl]
        rga_d = np.ascontiguousarray(
            rga_c.reshape(ck, P, HWC).transpose(1, 0, 2).reshape(P, ck * HWC))
        in_maps.append({
            "qt": qt_d, "kf": kf_d, "cvt": cvt_d, "rga": rga_d,
        })
    return in_maps


def assemble(results):
    fused = np.empty((B, C, H, W), dtype=np.float32)
    fused2 = fused.reshape(B, C, HW)
    ck = C // P
    for core in range(N_CORES):
        b, half = divmod(core, 2)
        o = np.asarray(results[core]["out"], dtype=np.float32)
        o = o.reshape(P, ck, HWC).transpose(1, 0, 2).reshape(C, HWC)
        fused2[b, :, half * HWC:(half + 1) * HWC] = o
    return fused


def kernel(rgb_features, chm_features, Wq, bq, Wk, bk, Wv, bv, gamma):
    nc = _get_nc()
    in_maps = make_in_maps(rgb_features, chm_features, Wq, bq, Wk, bk, Wv, bv,
                           gamma)
    res = run_bass_kernel_spmd(nc, in_maps, core_ids=list(range(N_CORES)))
    return assemble(res.results)


# revision 39
# speedup vs baseline: 1.0061x; 1.0026x over previous
"""Trainium2 Bass kernel for nn_CrossAttention (B=4, C=512, H=W=64, CQK=64).

Math (per batch b):
    Q = Wq @ rgb + bq                      [CQK, HW]
    K = Wk @ chm + bk                      [CQK, XY]
    S[hw, xy] = sum_o Q[o, hw] K[o, xy]    (xy = x*64 + y)
    P = softmax over y only (last 64-group of xy)
    att[c, hw] = sum_xy P[hw, xy] V[c, xy],  V = Wv @ chm + bv
    out = rgb + gamma * att

Sharding: 8 cores = 4 batches x 2 halves of the hw (query) axis; each core
computes its 2048-query slice of the attention map and attended output
against the full 4096-key/value domain of its batch. No collectives needed.

The small 1x1-conv GEMMs (Q/K/V projections; see sharding hint) are folded
into host-side input prep, exactly:
  - qt = Wq @ rgb + bq, kf = Wk @ chm + bk (f32 GEMMs, cast bf16).
  - chm' = (gamma*Wv) @ chm, pre-transposed into attend-weight tiles, so the
    device attend GEMM produces gamma*att directly.
  - bv contributes exactly 64*gamma*bv[c] per output pixel (softmax rows sum
    to 1 per (hw, x), 64 x-groups), folded into the residual rgb operand.
The quadratic attention compute (S = Q^T K, softmax, attend) runs on device.

Device dataflow per core (bf16 matmuls, f32 PSUM accumulate), per 128-row
query tile ("htile"):
  - S on PE ([128, 1024] PSUM tiles), exp on ACT -> E bf16.
  - Z via DVE pairwise-tree sum over y; reciprocal; 1/Z broadcast-expanded on
    GPSIMD so the DVE normalize multiply gets packed operands (2x bf16 mode);
    P^T via DMA xbar transpose.
  - Attend chains (32 accumulating matmuls, N=128 columns) interleaved `lag`
    htiles behind the S/softmax pipeline so the PE never idles (idle gaps
    reset the p-state ramp). DVE adds the f32 rgb residual; per-htile stores.
DMA: everything with late semaphore waits (transposes, residual loads,
stores) on the SP HWDGE ring; the early dependency-free attend-weight loads
on the ACT ring, which otherwise stays exp-only so DMA waits can never block
the exp stream at the ACT sequencer. DRAM layouts are pre-arranged so bulk
loads are contiguous per partition.
"""

import numpy as np
import ml_dtypes

import concourse.bass as bass
import concourse.mybir as mybir
import concourse.tile as tile
from concourse import bacc
from concourse.bass_utils import run_bass_kernel_spmd

P = 128
B, C, H, W = 4, 512, 64, 64
HW = H * W                # 4096
CQK = C // 8              # 64
N_CORES = 8
HWC = HW // 2             # hw rows per core (2048)

F32 = mybir.dt.float32
BF16 = mybir.dt.bfloat16
FP8 = mybir.dt.float8e4
DR = mybir.MatmulPerfMode.DoubleRow
ADD = mybir.AluOpType.add
MULT = mybir.AluOpType.mult
IDENT = mybir.ActivationFunctionType.Identity
EXP = mybir.ActivationFunctionType.Exp

BF16NP = ml_dtypes.bfloat16
FP8NP = ml_dtypes.float8_e4m3


def build_program(hwc=HWC, xy=HW, c=C, cqk=CQK, n_cores=N_CORES, lag=6,
                  direct_head=2, direct_tail=2):
    """Build the per-core Bass program. Returns a compiled Bacc module."""
    ck = c // P               # channel chunks (4)
    nb = hwc // 512           # hw blocks (4)
    nh = hwc // P             # hw tiles (16)
    xt = xy // P              # xy tiles (32)
    y = 64                    # softmax group size
    xg = xy // y              # x values (64)

    nc = bacc.Bacc("TRN2", target_bir_lowering=False, debug=False,
                   num_devices=n_cores)
    ld = nc.sync
    st = nc.scalar

    qtd = nc.dram_tensor("qt", [cqk, hwc], BF16, kind="ExternalInput")
    kfd = nc.dram_tensor("kf", [cqk, xy], BF16, kind="ExternalInput")
    # fp8 bytes shipped as uint8: the PJRT axon backend cannot compile
    # float8 I/O, so the dram tensor is uint8 and the AP is bitcast
    cvt = nc.dram_tensor("cvt", [P, ck * xt * P], mybir.dt.uint8,
                         kind="ExternalInput")
    rga = nc.dram_tensor("rga", [P, ck * hwc], F32, kind="ExternalInput")
    out = nc.dram_tensor("out", [P, ck * hwc], F32, kind="ExternalOutput")

    cvt_t = cvt.ap().bitcast(FP8).rearrange("p (m k j q) -> p m k j q",
                                            m=xt // 2, k=ck, j=2)
    rga_t = rga.ap().rearrange("p (k n) -> p k n", k=ck)
    out_t = out.ap().rearrange("p (k n) -> p k n", k=ck)

    with tile.TileContext(nc) as tc:
        with tc.tile_pool(name="pers", bufs=1) as pers:
            # one FIFO ring for everything: qt/kf first (S(0) blocks on
            # them), then the attend-weight chunks, so nothing can race
            # ahead of the critical first loads at the DMA engines
            qt = pers.tile([cqk, hwc], BF16)
            ld.dma_start(qt[:, 0:P], qtd.ap()[:, 0:P])
            kf = pers.tile([cqk, xy], BF16)
            ld.dma_start(kf[:, 0:xy // 2], kfd.ap()[:, 0:xy // 2])
            ld.dma_start(kf[:, xy // 2:], kfd.ap()[:, xy // 2:])
            ld.dma_start(qt[:, P:], qtd.ap()[:, P:])
            cvt_sb = pers.tile([P, xt // 2, ck, 2, P], FP8)
            for q4 in range(4):
                ld.dma_start(cvt_sb[:, 4 * q4:4 * (q4 + 1)],
                             cvt_t[:, 4 * q4:4 * (q4 + 1)])

            with tc.tile_pool(name="pmain", bufs=8) as pmain, \
                 tc.tile_pool(name="zpool", bufs=1) as zpool, \
                 tc.tile_pool(name="rzpool", bufs=3) as rzpool, \
                 tc.tile_pool(name="rzbpool", bufs=2) as rzbpool, \
                 tc.tile_pool(name="ptpool", bufs=4) as ptpool, \
                 tc.tile_pool(name="pt8pool", bufs=lag + 2) as pt8pool, \
                 tc.tile_pool(name="rgf", bufs=2) as rgf, \
                 tc.tile_pool(name="opool", bufs=3) as opool, \
                 tc.tile_pool(name="psS", bufs=6, space="PSUM") as psS, \
                 tc.tile_pool(name="psA", bufs=2, space="PSUM") as psA, \
                 nc.allow_low_precision(reason="softmax weights in bf16"):

                def softmax_s(h):
                    """S matmuls + exp chunks."""
                    p_sb = pmain.tile([P, xy], BF16, tag="p")
                    for s in range(xy // 512):
                        s_ps = psS.tile([P, 512], F32, tag="sps")
                        nc.tensor.matmul(
                            s_ps[:],
                            qt[:, P * h:P * (h + 1)],
                            kf[:, 512 * s:512 * (s + 1)],
                            start=True, stop=True)
                        nc.scalar.activation(
                            p_sb[:, 512 * s:512 * (s + 1)], s_ps[:], EXP)
                    return p_sb

                def softmax_z(h, p_sb):
                    """Z = sum over y (pairwise tree, bf16), then 1/Z."""
                    v3 = p_sb[:].rearrange("p (x y) -> p x y", y=y)
                    tcur = v3
                    w = y
                    while w > 1:
                        w //= 2
                        tnext = zpool.tile([P, xg, w], BF16, tag=f"z{w}")
                        nc.vector.tensor_tensor(
                            tnext[:], tcur[:, :, 0:w], tcur[:, :, w:2 * w],
                            ADD)
                        tcur = tnext
                    rz = rzpool.tile([P, xg, 1], BF16, tag="rz")
                    nc.vector.reciprocal(rz[:], tcur[:])
                    return p_sb, rz

                def softmax_back(h, p_sb, rz):
                    """Normalize, transpose, fp8 repack. Issued one round
                    after front(h): the ACT rzb expansion's wait (on recip)
                    is then already satisfied, so it can't head-of-line
                    block the next htile's exp stream at the ACT sequencer."""
                    v3 = p_sb[:].rearrange("p (x y) -> p x y", y=y)
                    if h % 2 == 0:
                        # expand 1/Z on ACT so the DVE multiply gets packed
                        # operands (2x bf16 mode); alternate htiles keep the
                        # whole normalize on DVE to balance ACT vs DVE
                        rzb = rzbpool.tile([P, xg, y], BF16, tag="rzb")
                        nc.scalar.activation(
                            rzb[:], rz[:].to_broadcast([P, xg, y]), IDENT)
                        nc.vector.tensor_tensor(v3, v3, rzb[:], MULT)
                    else:
                        nc.vector.tensor_tensor(
                            v3, v3, rz[:].to_broadcast([P, xg, y]), MULT)
                    ptb = ptpool.tile([P, xt, P], BF16, tag="ptb")
                    nc.sync.dma_start(ptb[:], p_sb[:], transpose=True)
                    # repack to fp8 DoubleRow pair layout:
                    # ptb8[p, m, j, n] = P^T[(m + 16j)*128 + p, n]
                    # GPSIMD in steady state; ACT for head/tail htiles where
                    # its exp stream is idle and GPSIMD serialization would
                    # otherwise pace the pipeline fill/drain
                    ptb8 = pt8pool.tile([P, xt // 2, 2, P], FP8, tag="ptb8")
                    src8 = ptb[:].rearrange("p (j m) q -> p m j q", j=2)
                    if h == 0 or h >= nh - 4:
                        nc.scalar.activation(ptb8[:], src8, IDENT)
                    else:
                        nc.gpsimd.tensor_copy(ptb8[:], src8)
                    return ptb8

                rg_blk = [None] * nb

                def attend_htile(g, ptb):
                    blk, ht = divmod(g, nb)
                    if ht == 0:
                        rg = rgf.tile([P, ck, 512], F32, tag="rg",
                                      name=f"rg{blk}")
                        ld.dma_start(rg[:],
                                     rga_t[:, :, 512 * blk:512 * (blk + 1)])
                        rg_blk[blk] = rg
                    rg = rg_blk[blk]
                    o_sb = opool.tile([P, ck, P], F32, tag="o")
                    cols = slice(P * ht, P * (ht + 1))
                    for ch in range(ck):
                        a_ps = psA.tile([P, P], F32, tag="aps")
                        for m in range(xt // 2):
                            nc.tensor.matmul(
                                a_ps[:], cvt_sb[:, m, ch], ptb[:, m],
                                start=(m == 0), stop=(m == xt // 2 - 1),
                                perf_mode=DR)
                        nc.vector.tensor_tensor(o_sb[:, ch], a_ps[:],
                                                rg[:, ch, cols], ADD)
                    ld.dma_start(out_t[:, :, P * g:P * (g + 1)], o_sb[:])

                # software pipeline, one stage per round:
                #   S/exp(h) | tree/recip(h-1) | normalize+transpose+
                #   repack(h-2) | ... | attend(h-lag)
                # so the ACT exp stream and the DVE tree stream each start a
                # round with all waits already satisfied
                sbufs = {}
                fronts = {}
                ptbs = {}
                for h in range(nh + 2):
                    if h < nh:
                        sbufs[h] = softmax_s(h)
                    if h >= 1 and h - 1 < nh:
                        fronts[h - 1] = softmax_z(h - 1, sbufs.pop(h - 1))
                    if h >= 2 and h - 2 < nh:
                        ptbs[h - 2] = softmax_back(h - 2, *fronts.pop(h - 2))
                    if h >= lag and h - lag < nh:
                        attend_htile(h - lag, ptbs.pop(h - lag))
                for g in range(nh + 2 - lag, nh):
                    attend_htile(g, ptbs.pop(g))

    nc.compile()
    return nc


_NC_CACHE = {}


def _get_nc():
    if "nc" not in _NC_CACHE:
        _NC_CACHE["nc"] = build_program()
    return _NC_CACHE["nc"]


def make_in_maps(rgb_features, chm_features, Wq, bq, Wk, bk, Wv, bv, gamma):
    rgb_features = np.asarray(rgb_features, dtype=np.float32)
    chm_features = np.asarray(chm_features, dtype=np.float32)
    Wq = np.asarray(Wq, dtype=np.float32)
    Wk = np.asarray(Wk, dtype=np.float32)
    Wv = np.asarray(Wv, dtype=np.float32)
    bq = np.asarray(bq, dtype=np.float32).reshape(CQK, 1)
    bk = np.asarray(bk, dtype=np.float32).reshape(CQK, 1)
    bv = np.asarray(bv, dtype=np.float32)
    g = float(np.asarray(gamma).reshape(-1)[0])

    ck = C // P
    xt = HW // P
    # softmax rows sum to 1 per (hw, x); summing over the 64 x's makes the
    # bias term contribute exactly 64*gamma*bv[c] to every output pixel.
    rgb_adj = rgb_features + (64.0 * g * bv)[None, :, None, None]
    gwv = g * Wv

    in_maps = []
    per_batch = {}
    for core in range(N_CORES):
        b, half = divmod(core, 2)
        if b not in per_batch:
            chm_b = chm_features[b].reshape(C, HW)
            kf_d = (Wk @ chm_b + bk).astype(BF16NP)      # [CQK, XY]
            # chm' = (gamma Wv) @ chm, pre-transposed to the attend-weight
            # tile layout: cvt[p, k, t, q] = chm'[k*128+q, t*128+p]
            chmp = (gwv @ chm_b).astype(FP8NP)           # [C, XY]
            # cvt[p, m, ch, j, q] = chm'[ch*128+q, (m + 16j)*128 + p]
            A = chmp.reshape(ck, P, 2, xt // 2, P)       # [ch, q, j, m, p]
            cvt_d = np.ascontiguousarray(
                A.transpose(4, 3, 0, 2, 1).reshape(P, ck * xt * P))
            cvt_d = cvt_d.view(np.uint8)
            per_batch[b] = (kf_d, cvt_d)
        kf_d, cvt_d = per_batch[b]

        sl = slice(half * HWC, (half + 1) * HWC)
        rgb_c = rgb_features[b].reshape(C, HW)[:, sl]
        qt_d = (Wq @ rgb_c + bq).astype(BF16NP)          # [CQK, HWC]
        rga_c = rgb_adj[b].reshape(C, HW)[:, sl]
        rga_d = np.ascontiguousarray(
            rga_c.reshape(ck, P, HWC).transpose(1, 0, 2).reshape(P, ck * HWC))
        in_maps.append({
            "qt": qt_d, "kf": kf_d, "cvt": cvt_d, "rga": rga_d,
        })
    return in_maps


def assemble(results):
    fused = np.empty((B, C, H, W), dtype=np.float32)
    fused2 = fused.reshape(B, C, HW)
    ck = C // P
    for core in range(N_CORES):
        b, half = divmod(core, 2)
        o = np.asarray(results[core]["out"], dtype=np.float32)
        o = o.reshape(P, ck, HWC).transpose(1, 0, 2).reshape(C, HWC)
        fused2[b, :, half * HWC:(half + 1) * HWC] = o
    return fused


def kernel(rgb_features, chm_features, Wq, bq, Wk, bk, Wv, bv, gamma):
    nc = _get_nc()
    in_maps = make_in_maps(rgb_features, chm_features, Wq, bq, Wk, bk, Wv, bv,
                           gamma)
    res = run_bass_kernel_spmd(nc, in_maps, core_ids=list(range(N_CORES)))
    return assemble(res.results)
